# revision 101
# baseline (speedup 1.0000x reference)
"""DeepSeek-V3 MLA forward (B=1, S=2048, D=4096, H=32) on 8 TRN2 NeuronCores.

Sharding: the low-rank a-projections (the dominant replicated cost in the
previous version) are SEQUENCE-sharded: each core computes qa / compressed-kv
/ rope-key only for its 256-token slice, then
  - an AllGather redistributes the (normalized) compressed kv + rope key
    (everyone needs all keys), and
  - the q b-projection is computed locally for ALL 32 heads on the 256-token
    slice (same FLOPs as 4 heads x full seq) and an AllToAll re-shards it to
    tensor-parallel-over-heads layout (each core gets its 4 heads over the
    full sequence) -- much cheaper on the wire than AllGathering qa.
Attention, kv b-projection and out-proj stay head-sharded (4 heads/core);
the post-out-proj all-reduce is done host-side while unsharding.

Layout strategy: all activations live feature-major (x^T: [feat(part), seq
(free)]) so every matmul is lhsT=W-chunk, rhs=x^T with no on-device
transposes. Attention computes scores TRANSPOSED (sT[rk, rq]); softmax's
denominator is accumulated on the DVE (elementwise adds of the exp tiles)
with a single ones-matmul column-reduce per (head, q-tile), the exp is a
plain ACT pass, and P@V consumes expT directly as the moving operand.
RMSNorm scales are applied to qa / c BEFORE the collectives (they commute
with the feature contraction), so no statistics travel between cores.

Numerics: bf16 operands everywhere on the PE (full rate), fp32 accumulation
in PSUM; out-proj partials are written bf16 and summed on the host in fp32.
"""

import math
from dataclasses import dataclass

import ml_dtypes
import numpy as np

import concourse.bass as bass
import concourse.mybir as mybir
import concourse.tile as tile
from concourse import bacc
from concourse.bass_utils import run_bass_kernel_spmd

F32 = mybir.dt.float32
F32R = mybir.dt.float32r
BF16 = mybir.dt.bfloat16
F8 = mybir.dt.float8e4
AF = mybir.ActivationFunctionType
BF16NP = ml_dtypes.bfloat16

N_CORES = 8
EPS = 1e-6
THETA = 10000.0


@dataclass(frozen=True)
class Cfg:
    S: int = 2048
    D: int = 4096
    QR: int = 1536      # q lora rank
    KVR: int = 512      # kv lora rank
    H: int = 32         # total heads
    HPC: int = 4        # heads per core
    NOPE: int = 128
    ROPE: int = 64
    VD: int = 128

    @property
    def QD(self):
        return self.NOPE + self.ROPE

    @property
    def SL(self):          # per-core sequence slice
        return self.S // N_CORES

    @property
    def DCH(self):
        return self.D // 128

    @property
    def QRCH(self):
        return self.QR // 128

    @property
    def KVCH(self):
        return self.KVR // 128

    @property
    def AM(self):          # a-proj m-chunks: q rank + kv rank + 1 rope(64pad)
        return self.QRCH + self.KVCH + 1

    @property
    def NQT(self):         # 512-wide query tiles
        return self.S // 512

    @property
    def NKI(self):         # 128-wide key blocks
        return self.S // 128

    @property
    def CKROWS(self):      # gathered block rows: c (KVR) + krope (ROPE)
        return self.KVR + self.ROPE

    @property
    def QXCH(self):        # q chunks per dst: HPC nope + HPC/2 rope pairs
        return self.HPC + self.HPC // 2


FULL = Cfg()


# --------------------------------------------------------------------------
# host-side input preparation
# --------------------------------------------------------------------------

def _rope_perm(rope):
    # deepseek interleave: xp = concat(x[0::2], x[1::2]) acting on rope dims
    return np.concatenate([np.arange(0, rope, 2), np.arange(1, rope, 2)])


def prep_inputs(cfg, hidden_states, Wq_a, q_a_ln_w, Wq_b, Wkv_a, kv_a_ln_w,
                Wkv_b, Wo):
    c = cfg
    hs = np.asarray(hidden_states, np.float32).reshape(c.S, c.D)
    Wq_a = np.asarray(Wq_a, np.float32)
    Wq_b = np.asarray(Wq_b, np.float32)
    Wkv_a = np.asarray(Wkv_a, np.float32)
    Wkv_b = np.asarray(Wkv_b, np.float32)
    Wo = np.asarray(Wo, np.float32)
    q_a_ln_w = np.asarray(q_a_ln_w, np.float32)
    kv_a_ln_w = np.asarray(kv_a_ln_w, np.float32)

    # hidden^T sliced per core, layout [128(d-sub), DCH, SL]: one DMA
    hT = np.ascontiguousarray(hs.T)                      # [D, S]
    hT = hT.reshape(c.DCH, 128, N_CORES, c.SL)
    hT = np.ascontiguousarray(hT.transpose(2, 1, 0, 3)).astype(BF16NP)

    # combined a-proj weight, padded to AM*128 cols: [AM, DCH, 128, 128].
    # qa chunks first (m 0..QRCH-1), then c (m QRCH..QRCH+KVCH-1), then the
    # shared-key rope chunk (64 real rows). The rope columns of Wkv_a get the
    # deepseek interleave permutation folded in.
    perm_a = _rope_perm(c.ROPE)
    Wkv_a_p = np.concatenate(
        [Wkv_a[:, :c.KVR], Wkv_a[:, c.KVR:][:, perm_a]], axis=1)
    wa = np.concatenate([Wq_a, Wkv_a_p], axis=1)         # [D, QR+KVR+ROPE]
    pad = c.AM * 128 - wa.shape[1]
    wa = np.pad(wa, ((0, 0), (0, pad)))
    # layout [AM, 128(d-sub), DCH*128]: one fully-contiguous DMA per m-chunk
    wa = wa.reshape(c.DCH, 128, c.AM, 128)
    wa = np.ascontiguousarray(wa.transpose(2, 1, 0, 3)).astype(BF16NP)
    wa = wa.reshape(c.AM, 128, c.DCH * 128)

    # q b-projection for ALL heads, columns grouped by destination core,
    # split into the nope part (dst d: nope(head 4d..4d+3), 512 cols) and
    # the rope part (dst d: rope-pair(4d,4d+1) | rope-pair(4d+2,4d+3),
    # 256 cols) -- exchanged by two separate AllToAlls
    qd, nope, rope, vd = c.QD, c.NOPE, c.ROPE, c.VD
    scale = qd ** (-0.5)
    wqb_all = (Wq_b * q_a_ln_w[:, None]).reshape(c.QR, c.H, qd) * scale
    perm = _rope_perm(rope)
    wqb_nope = wqb_all[:, :, :nope]                      # [QR, H, 128]
    wqb_rope = wqb_all[:, :, nope:][:, :, perm]          # [QR, H, 64]
    wqbn = np.ascontiguousarray(
        wqb_nope.reshape(c.QR, c.H * nope)
        .reshape(c.QRCH, 128, c.H * nope)).astype(BF16NP)
    wqbr = np.ascontiguousarray(
        wqb_rope.reshape(c.QR, c.H * rope)
        .reshape(c.QRCH, 128, c.H * rope)).astype(BF16NP)

    wkv_all = (Wkv_b * kv_a_ln_w[:, None]).reshape(c.KVR, c.H, nope + vd)

    # rotary tables, feature-major, replicated to 128 rows, sliced per core
    inv_freq = 1.0 / (THETA ** (np.arange(0, rope, 2, np.float32) / rope))
    freqs = np.outer(np.arange(c.S, dtype=np.float32), inv_freq)  # [S, 32]
    cosT = np.tile(np.cos(freqs).T, (4, 1)).astype(BF16NP)        # [128, S]
    sinT = np.tile(np.sin(freqs).T, (4, 1)).astype(BF16NP)
    # rotate-half as a PE matmul: rot = R @ x with R block-diag over two
    # 64-row rope groups, R = [[0, -I32], [I32, 0]] per group. lhsT = R.T.
    R = np.zeros((128, 128), np.float32)
    for blk in (0, 64):
        for i in range(32):
            R[blk + i, blk + i + 32] = -1.0
            R[blk + i + 32, blk + i] = 1.0
    rotT = np.ascontiguousarray(R.T)

    # diagonal-tile masks: mask01[j][r, q] = 1 if 128*j + r <= q
    j = np.arange(4)[:, None, None]
    r = np.arange(128)[None, :, None]
    q = np.arange(512)[None, None, :]
    mask01 = ((128 * j + r) <= q).astype(BF16NP)

    shared = {
        "wa": wa,
        "wqbn": wqbn,
        "wqbr": wqbr,
        "rotT": rotT,
        "ones_f": np.ones((128, 128), np.float32),
        "mask01": mask01,
    }
    in_maps = []
    for core in range(N_CORES):
        hsel = np.arange(core * c.HPC, (core + 1) * c.HPC)
        wkb_c = np.ascontiguousarray(
            wkv_all[:, hsel, :nope].reshape(c.KVCH, 128, c.HPC * nope)
        ).astype(BF16NP)
        wv_c = np.ascontiguousarray(
            wkv_all[:, hsel, nope:].reshape(c.KVCH, 128, c.HPC * vd)
        ).astype(BF16NP)
        wo_c = np.ascontiguousarray(
            Wo.reshape(c.H, vd, c.D)[hsel]).astype(BF16NP)
        sl = slice(core * c.SL, (core + 1) * c.SL)
        in_maps.append(dict(
            shared, hT=hT[core], wkb=wkb_c, wv=wv_c, wo=wo_c,
            cosT=np.ascontiguousarray(cosT[:, sl]),
            sinT=np.ascontiguousarray(sinT[:, sl])))
    return in_maps


# --------------------------------------------------------------------------
# kernel builder
# --------------------------------------------------------------------------

def build(cfg):
    c = cfg
    nc = bacc.Bacc("TRN2", target_bir_lowering=False, debug=False,
                   num_devices=N_CORES)

    hT_d = nc.declare_dram_parameter("hT", [128, c.DCH, c.SL], BF16, isOutput=False)
    wa_d = nc.declare_dram_parameter("wa", [c.AM, 128, c.DCH * 128], BF16, isOutput=False)
    wqbn_d = nc.declare_dram_parameter("wqbn", [c.QRCH, 128, c.H * c.NOPE], BF16, isOutput=False)
    wqbr_d = nc.declare_dram_parameter("wqbr", [c.QRCH, 128, c.H * c.ROPE], BF16, isOutput=False)
    wkb_d = nc.declare_dram_parameter("wkb", [c.KVCH, 128, c.HPC * c.NOPE], BF16, isOutput=False)
    wv_d = nc.declare_dram_parameter("wv", [c.KVCH, 128, c.HPC * c.VD], BF16, isOutput=False)
    wo_d = nc.declare_dram_parameter("wo", [c.HPC, 128, c.D], BF16, isOutput=False)
    cos_d = nc.declare_dram_parameter("cosT", [128, c.SL], BF16, isOutput=False)
    sin_d = nc.declare_dram_parameter("sinT", [128, c.SL], BF16, isOutput=False)
    rot_d = nc.declare_dram_parameter("rotT", [128, 128], F32R, isOutput=False)
    ones_d = nc.declare_dram_parameter("ones_f", [128, 128], F32R, isOutput=False)
    mask_d = nc.declare_dram_parameter("mask01", [4, 128, 512], BF16, isOutput=False)
    out_d = nc.declare_dram_parameter("outT", [c.DCH, 128, c.S], BF16, isOutput=True)

    # collective bounce buffers (inputs Local, outputs Shared)
    ck_in = nc.dram_tensor("ck_in", [c.CKROWS, c.SL], BF16)
    ck_out = nc.dram_tensor("ck_out", [N_CORES, c.CKROWS, c.SL], BF16,
                            addr_space="Shared")
    # q travels fp8e4m3: halves the AllToAll wire time; verified to keep
    # the final absmax rel-err ~8e-3 (gate 2e-2). The rope part goes in a
    # separate (earlier) AllToAll so the exchange pipelines with the
    # nope b-projection.
    qxn_in = nc.dram_tensor("qxn_in", [N_CORES, c.HPC, 128, c.SL], F8)
    qxn_out = nc.dram_tensor("qxn_out", [N_CORES, c.HPC, 128, c.SL], F8)
    qxr_in = nc.dram_tensor("qxr_in", [N_CORES, c.HPC // 2, 128, c.SL], F8)
    qxr_out = nc.dram_tensor("qxr_out", [N_CORES, c.HPC // 2, 128, c.SL], F8)
    GROUPS = [list(range(N_CORES))]

    with tile.TileContext(nc) as tc:
        with tc.tile_pool(name="persist", bufs=1) as pp:
            # persistent tiles (DMAs issued after the first compute loads)
            cos_sb = pp.tile([128, c.SL], BF16, name="cos_sb")
            sin_sb = pp.tile([128, c.SL], BF16, name="sin_sb")
            rot_sb = pp.tile([128, 128], F32R, name="rot_sb")
            ones_sb = pp.tile([128, 128], F32R, name="ones_sb")
            ones_col_f = ones_sb[:, 0:1]
            ones_row_f = ones_sb[0:1, :]
            ones_col_b = pp.tile([128, 1], BF16, name="ones_col_b")
            nc.vector.memset(ones_col_b[:], 1.0)

            # qa stays unnormalized (f32r) for the q b-projection; the
            # rms scale commutes through the contraction and is folded
            # into the staging writes (cossq/sinsq for the rope part)
            cossq = pp.tile([128, c.SL], F32, name="cossq")
            sinsq = pp.tile([128, c.SL], F32, name="sinsq")

            # ---------------- phase A: a-projections (seq slice) -------
            with tc.tile_pool(name="pA", bufs=1) as pA, \
                 tc.tile_pool(name="pA_w", bufs=5) as pAw, \
                 tc.tile_pool(name="pA_ev", bufs=4) as pAe, \
                 tc.tile_pool(name="pA_ps", bufs=3, space="PSUM") as psA, \
                 tc.tile_pool(name="pA_ps1", bufs=1, space="PSUM") as psA1:
                def load_wa(m, split=1):
                    wa_sb = pAw.tile([128, c.DCH * 128], BF16, name="wa_sb")
                    step = c.DCH * 128 // split
                    for jj in range(split):
                        nc.sync.dma_start(
                            wa_sb[:, jj * step:(jj + 1) * step],
                            wa_d.ap()[m, :, jj * step:(jj + 1) * step])
                    return wa_sb

                # krope + c chunks first so the AllGather can launch early,
                # then the qa chunks. krope leads so its rope vector chain
                # overlaps the c matmuls.
                m_order = ([c.AM - 1] + list(range(c.QRCH, c.AM - 1))
                           + list(range(c.QRCH)))
                wa_next = load_wa(m_order[0], split=4)
                # cos/sin are tiny and feed the krope chain (the FIRST
                # m-chunk): load them before the bulky hT stream so the
                # rope rotation never stalls on them
                nc.sync.dma_start(cos_sb[:], cos_d.ap())
                nc.sync.dma_start(sin_sb[:], sin_d.ap())
                nc.sync.dma_start(rot_sb[:], rot_d.ap())
                hT_all = pA.tile([128, c.DCH, c.SL], BF16, name="hT_all")
                HCH = c.DCH // 8
                for j in range(8):
                    nc.sync.dma_start(
                        hT_all[:, j * HCH:(j + 1) * HCH, :],
                        hT_d.ap()[:, j * HCH:(j + 1) * HCH, :])
                hT_sb = [hT_all[:, k, :] for k in range(c.DCH)]
                nc.sync.dma_start(ones_sb[:], ones_d.ap())
                ssq = psA1.tile([1, c.SL], F32, name="ssq_q")
                ssc = psA1.tile([1, c.SL], F32, name="ssq_c")
                qa_raw = []
                c_raw = []

                def rms_bcast(ps1, denom):
                    # rsqrt(mean(x^2)+eps) row -> broadcast to 128 rows
                    t = pAe.tile([1, c.SL], F32, name="rms_t")
                    nc.vector.tensor_scalar(
                        t[:], ps1[:], 1.0 / denom, EPS,
                        mybir.AluOpType.mult, mybir.AluOpType.add)
                    st = pAe.tile([1, c.SL], F32, name="rms_st")
                    nc.scalar.activation(st[:], t[:], AF.Sqrt)
                    rc = pAe.tile([1, c.SL], F32R, name="rms_rc")
                    with nc.allow_low_precision(reason="fp32r for PE bcast"):
                        nc.vector.reciprocal(rc[:], st[:])
                    bc_ps = psA.tile([128, c.SL], F32, name="psA")
                    nc.tensor.matmul(bc_ps[:], ones_row_f, rc[:])
                    bc = pA.tile([128, c.SL], F32, name=f"bc_{denom}")
                    nc.vector.tensor_copy(bc[:], bc_ps[:])
                    return bc

                for idx, m in enumerate(m_order):
                    wa_sb = wa_next
                    if idx + 1 < len(m_order):
                        wa_next = load_wa(m_order[idx + 1])
                    ps = psA.tile([128, c.SL], F32, name="psA")
                    for k in range(c.DCH):
                        nc.tensor.matmul(
                            ps[:], wa_sb[:, k * 128:(k + 1) * 128], hT_sb[k],
                            start=(k == 0), stop=(k == c.DCH - 1))
                    if m < c.QRCH:
                        # bf16: the q b-proj consumes this as the moving
                        # operand against bf16 weights (the BIR verifier
                        # rejects f32r mixed with other dtypes)
                        ev = pp.tile([128, c.SL], BF16, name=f"qa_raw_{m}")
                        nc.vector.tensor_copy(ev[:], ps[:])
                        qa_raw.append(ev)
                        x2 = pAe.tile([128, c.SL], F32R, name="x2")
                        nc.vector.tensor_mul(x2[:], ev[:], ev[:])
                        nc.tensor.matmul(ssq[:], ones_col_f, x2[:],
                                         start=(m == 0),
                                         stop=(m == c.QRCH - 1))
                    elif m < c.QRCH + c.KVCH:
                        mc = m - c.QRCH
                        ev = pA.tile([128, c.SL], F32R, name=f"c_raw_{mc}")
                        nc.vector.tensor_copy(ev[:], ps[:])
                        c_raw.append(ev)
                        x2 = pAe.tile([128, c.SL], F32R, name="x2")
                        nc.vector.tensor_mul(x2[:], ev[:], ev[:])
                        nc.tensor.matmul(ssc[:], ones_col_f, x2[:],
                                         start=(mc == 0),
                                         stop=(mc == c.KVCH - 1))
                        if mc == c.KVCH - 1:
                            # c norm + gather launch
                            bc_sc = rms_bcast(ssc, c.KVR)
                            for j in range(c.KVCH):
                                cn = pAe.tile([128, c.SL], BF16,
                                              name=f"cn_{j % 2}")
                                ceng = (nc.vector if j % 2 == 0
                                        else nc.gpsimd)
                                ceng.tensor_mul(cn[:], c_raw[j][:],
                                                bc_sc[:])
                                nc.sync.dma_start(
                                    ck_in.ap()[j * 128:(j + 1) * 128, :],
                                    cn[:])
                            nc.gpsimd.collective_compute(
                                "AllGather", mybir.AluOpType.bypass,
                                replica_groups=GROUPS,
                                ins=[ck_in.ap().opt()],
                                outs=[ck_out.ap().opt()])
                    else:
                        # shared rope key: rows 0:64, rope applied in place
                        kr = pAe.tile([64, c.SL], F32R, name="kr")
                        nc.vector.tensor_copy(kr[:], ps[0:64, :])
                        rps = psA.tile([64, c.SL], F32, name="rot_ps")
                        nc.tensor.matmul(rps[:], rot_sb[0:64, 0:64], kr[:])
                        rk = pAe.tile([64, c.SL], F32, name="rk")
                        nc.vector.tensor_copy(rk[:], rps[:])
                        a = pAe.tile([64, c.SL], F32, name="ra")
                        b = pAe.tile([64, c.SL], F32, name="rb")
                        nc.vector.tensor_mul(a[:], kr[:], cos_sb[0:64, :])
                        nc.vector.tensor_mul(b[:], rk[:], sin_sb[0:64, :])
                        kro = pAe.tile([64, c.SL], BF16, name="kro")
                        nc.vector.tensor_add(kro[:], a[:], b[:])
                        nc.sync.dma_start(
                            ck_in.ap()[c.KVR:c.CKROWS, :], kro[:])
                # qa scale deferred into the staging writes; precombine
                # it with the rotary tables for the rope part
                qa_n = qa_raw
                bc_sq = rms_bcast(ssq, c.QR)
                nc.vector.tensor_mul(cossq[:], cos_sb[:], bc_sq[:])
                nc.vector.tensor_mul(sinsq[:], sin_sb[:], bc_sq[:])

            # ---------------- B-phase residents, DMAs issued early ------
            # (on the Pool queue so they order after the AllGather without
            # blocking SP, which streams the q b-proj weights)
            pBC_cm = tc.tile_pool(name="pBC", bufs=1)
            pBC = pBC_cm.__enter__()
            knopeT = [pBC.tile([128, c.S], BF16, name=f"knopeT_{m}")
                      for m in range(c.HPC)]
            v_sb = [pBC.tile([128, c.HPC * c.VD], BF16, name=f"v_sb_{ki}")
                    for ki in range(c.NKI)]
            # key-rope expanded to 128 rows with the other head-half zeroed,
            # so the per-head rope score matmul can take the full 128-row
            # qrope chunk as rhs (matmul requires matching base partitions)
            krope2 = [pBC.tile([128, c.S], BF16, name=f"krope2_{par}")
                      for par in range(2)]
            pB_cm = tc.tile_pool(name="pB", bufs=1)
            pB = pB_cm.__enter__()
            wkb_sb = [pB.tile([128, c.HPC * c.NOPE], BF16, name=f"wkb_{kc}")
                      for kc in range(c.KVCH)]
            wv_sb = [pB.tile([128, c.HPC * c.VD], BF16, name=f"wv_{kc}")
                     for kc in range(c.KVCH)]
            wo_sb = [pBC.tile([128, c.D], BF16, name=f"wo_{k}")
                     for k in range(c.HPC)]
            mask_sb = [pBC.tile([128, 512], BF16, name=f"mask_{j}")
                       for j in range(4)]
            c_T = [pB.tile([128, c.S], BF16, name=f"c_T_{kc}")
                   for kc in range(c.KVCH)]
            nc.vector.memset(krope2[0][64:128, :], 0.0)
            nc.vector.memset(krope2[1][0:64, :], 0.0)

            def issue_b_loads():
                # issued AFTER the first AllToAll on the Pool queue: the
                # SWDGE descriptor generation (~1 us per DMA) then overlaps
                # the collective's transfer instead of delaying its launch
                kr_src = (ck_out.ap()[:, c.KVR:c.CKROWS, :]
                          .rearrange("s p c -> p s c"))
                nc.gpsimd.dma_start(
                    krope2[0][0:64, :]
                    .rearrange("p (s c) -> p s c", s=N_CORES), kr_src)
                nc.gpsimd.dma_start(
                    krope2[1][64:128, :]
                    .rearrange("p (s c) -> p s c", s=N_CORES), kr_src)
                for kc in range(c.KVCH):
                    nc.gpsimd.dma_start(
                        c_T[kc][:].rearrange("p (s c) -> p s c", s=N_CORES),
                        ck_out.ap()[:, kc * 128:(kc + 1) * 128, :]
                        .rearrange("s p c -> p s c"))
                    nc.gpsimd.dma_start(wkb_sb[kc][:], wkb_d.ap()[kc])
                    nc.gpsimd.dma_start(wv_sb[kc][:], wv_d.ap()[kc])
                for k in range(c.HPC):
                    nc.gpsimd.dma_start(wo_sb[k][:], wo_d.ap()[k])
                for j in range(4):
                    nc.gpsimd.dma_start(mask_sb[j][:], mask_d.ap()[j])

            # ---------------- phase Aq: q b-proj (all heads, own slice) -
            # rope pass first: its (small) AllToAll fires while the nope
            # pass still computes, so both exchanges pipeline with the PE
            with tc.tile_pool(name="pQ_w", bufs=3) as pQw, \
                 tc.tile_pool(name="pQ_ev", bufs=4) as pQe, \
                 tc.tile_pool(name="pQ_ps", bufs=6, space="PSUM") as psQ, \
                 tc.tile_pool(name="pQ_psr", bufs=2, space="PSUM") as psQr:
                NBLK = c.HPC * 128         # 512 nope cols per destination
                for d in range(N_CORES):
                    wqn_sb = pQw.tile([128, c.QRCH, NBLK], BF16, name="wqn_sb")
                    KH = c.QRCH // 2
                    for jh in range(2):
                        nc.sync.dma_start(
                            wqn_sb[:, jh * KH:(jh + 1) * KH, :],
                            wqbn_d.ap()[jh * KH:(jh + 1) * KH, :,
                                        d * NBLK:(d + 1) * NBLK]
                            .rearrange("k p c -> p k c"))
                    qstn = pQw.tile([128, c.HPC, c.SL], F8, name="qstn")
                    # two interleaved accumulation groups: the PE never
                    # waits on a group's trailing psum-drain chain
                    for mm0 in range(0, c.HPC, 2):
                        pses = [psQ.tile([128, c.SL], F32, name="psQ")
                                for _ in range(2)]
                        for k in range(c.QRCH):
                            for g in range(2):
                                nc.tensor.matmul(
                                    pses[g][:],
                                    wqn_sb[:, k,
                                           (mm0 + g) * 128:(mm0 + g + 1) * 128],
                                    qa_n[k][:], start=(k == 0),
                                    stop=(k == c.QRCH - 1))
                        for g in range(2):
                            nc.vector.tensor_mul(qstn[:, mm0 + g, :],
                                                 pses[g][:], bc_sq[:])
                    nc.scalar.dma_start(
                        qxn_in.ap()[d].rearrange("m p c -> p m c"), qstn[:])
                nc.gpsimd.collective_compute(
                    "AllToAll", mybir.AluOpType.bypass,
                    replica_groups=GROUPS,
                    ins=[qxn_in.ap().opt()],
                    outs=[qxn_out.ap().opt()])
                issue_b_loads()
                RBLK = c.HPC // 2 * 128    # 256 rope cols per destination
                for d in range(N_CORES):
                    wqr_sb = pQw.tile([128, c.QRCH, RBLK], BF16, name="wqr_sb")
                    nc.sync.dma_start(
                        wqr_sb[:],
                        wqbr_d.ap()[:, :, d * RBLK:(d + 1) * RBLK]
                        .rearrange("k p c -> p k c"))
                    qstr = pQw.tile([128, c.HPC // 2, c.SL], F8, name="qstr")
                    pses = [psQ.tile([128, c.SL], F32, name="psQ")
                            for _ in range(2)]
                    for k in range(c.QRCH):
                        for g in range(2):
                            nc.tensor.matmul(
                                pses[g][:],
                                wqr_sb[:, k, g * 128:(g + 1) * 128],
                                qa_n[k][:], start=(k == 0),
                                stop=(k == c.QRCH - 1))
                    for g in range(2):
                        # lean chain: the sin-multiply reads the rotation
                        # psum directly (saves a copy); rps lives in its
                        # own pool to decouple from the main psum rotation
                        ro = pQe.tile([128, c.SL], F32R, name="ro")
                        nc.vector.tensor_copy(ro[:], pses[g][:])
                        rps = psQr.tile([128, c.SL], F32, name="psQr")
                        nc.tensor.matmul(rps[:], rot_sb[:], ro[:])
                        a = pQe.tile([128, c.SL], F32, name="qra")
                        b = pQe.tile([128, c.SL], F32, name="qrb")
                        nc.vector.tensor_mul(a[:], ro[:], cossq[:])
                        nc.vector.tensor_mul(b[:], rps[:], sinsq[:])
                        nc.vector.tensor_add(qstr[:, g, :], a[:], b[:])
                    # issue from the idle ACT queue: SP stays free to
                    # stream the nope-pass weights meanwhile
                    nc.scalar.dma_start(
                        qxr_in.ap()[d].rearrange("m p c -> p m c"), qstr[:])
                nc.gpsimd.collective_compute(
                    "AllToAll", mybir.AluOpType.bypass,
                    replica_groups=GROUPS,
                    ins=[qxr_in.ap().opt()],
                    outs=[qxr_out.ap().opt()])

            # ---------------- phase B: kv b-projection -----------------
            with tc.tile_pool(name="pB_ps", bufs=3, space="PSUM") as psB:
                for m in range(c.HPC):
                    for n in range(c.S // 512):
                        ps = psB.tile([128, 512], F32, name="psB")
                        for kc in range(c.KVCH):
                            nc.tensor.matmul(
                                ps[:], wkb_sb[kc][:, m * 128:(m + 1) * 128],
                                c_T[kc][:, n * 512:(n + 1) * 512],
                                start=(kc == 0), stop=(kc == c.KVCH - 1))
                        nc.vector.tensor_copy(
                            knopeT[m][:, n * 512:(n + 1) * 512], ps[:])
                for ki in range(c.NKI):
                    ps = psB.tile([128, c.HPC * c.VD], F32, name="psB")
                    for kc in range(c.KVCH):
                        nc.tensor.matmul(
                            ps[:],
                            c_T[kc][:, ki * 128:(ki + 1) * 128],
                            wv_sb[kc][:], start=(kc == 0),
                            stop=(kc == c.KVCH - 1))
                    nc.vector.tensor_copy(v_sb[ki][:], ps[:])
            pB_cm.__exit__(None, None, None)

            # ---------------- phase C: attention + out-proj ------------
            with tc.tile_pool(name="pC", bufs=1) as pC, \
                 tc.tile_pool(name="pC2", bufs=2) as pC2, \
                 tc.tile_pool(name="pCe", bufs=2) as pCe, \
                 tc.tile_pool(name="pCd", bufs=2) as pCd, \
                 tc.tile_pool(name="pCx", bufs=6) as pCx, \
                 tc.tile_pool(name="pC_mm", bufs=2, space="PSUM") as psM, \
                 tc.tile_pool(name="pC_sT", bufs=3, space="PSUM") as psT, \
                 tc.tile_pool(name="pC_oT", bufs=2, space="PSUM") as psO, \
                 tc.tile_pool(name="pC_den", bufs=1, space="PSUM") as psD:
                for qi in range(c.NQT):
                    q0 = qi * 512
                    qnopeT = [pC2.tile([128, 512], F8, name=f"qnopeT_{m}")
                              for m in range(c.HPC)]
                    qrope_ch = [pC2.tile([128, 512], F8, name=f"qrope_{j}")
                                for j in range(c.HPC // 2)]
                    NS = 512 // c.SL
                    for m in range(c.HPC):
                        nc.sync.dma_start(
                            qnopeT[m][:].rearrange("p (s c) -> p s c", s=NS),
                            qxn_out.ap()[NS * qi:NS * (qi + 1), m]
                            .rearrange("s p c -> p s c"))
                    for j in range(c.HPC // 2):
                        nc.sync.dma_start(
                            qrope_ch[j][:].rearrange("p (s c) -> p s c", s=NS),
                            qxr_out.ap()[NS * qi:NS * (qi + 1), j]
                            .rearrange("s p c -> p s c"))

                    oT_sb = [pC2.tile([128, 512], BF16, name=f"oT_{h}")
                             for h in range(c.HPC)]
                    nki = 4 * (qi + 1)
                    for h in range(c.HPC):
                        oT_ps = psO.tile([128, 512], F32, name="psO")
                        # two bf16 denominator accumulators on DVE (2x DVE
                        # rate, halved serial chains; the 128-deep column
                        # reduce happens in fp32 on the PE afterwards)
                        den_acc = [pCd.tile([128, 512], BF16, name=f"den_{p}")
                                   for p in range(2)]
                        for ki in range(nki):
                            j = ki - (nki - 4)
                            # diagonal blocks j>=1: columns below 128*j are
                            # fully causal-masked -- compute the valid
                            # column subrange only (triangular tiling)
                            q0c = 128 * j if j > 0 else 0
                            sl = slice(q0c, 512)
                            sT_ps = psT.tile([128, 512], F32, name="psT")
                            nc.tensor.matmul(
                                sT_ps[:, sl],
                                knopeT[h][:, ki * 128:(ki + 1) * 128],
                                qnopeT[h][:, sl], start=True, stop=False)
                            nc.tensor.matmul(
                                sT_ps[:, sl],
                                krope2[h % 2][:, ki * 128:(ki + 1) * 128],
                                qrope_ch[h // 2][:, sl], start=False,
                                stop=True)
                            ex = pCx.tile([128, 512], BF16, name="expT")
                            nc.scalar.activation(ex[:, sl], sT_ps[:, sl],
                                                 AF.Exp)
                            if j >= 0:
                                nc.vector.tensor_mul(ex[:, sl], ex[:, sl],
                                                     mask_sb[j][:, sl])
                            eng = nc.vector
                            da = den_acc[ki % 2]
                            if ki < 2:
                                if q0c == 0:
                                    eng.tensor_copy(da[:], ex[:])
                                else:
                                    eng.memset(da[:], 0.0)
                                    eng.tensor_add(da[:, sl], da[:, sl],
                                                   ex[:, sl])
                            else:
                                eng.tensor_add(da[:, sl], da[:, sl],
                                               ex[:, sl])
                            nc.tensor.matmul(
                                oT_ps[:, sl],
                                v_sb[ki][:, h * c.VD:(h + 1) * c.VD],
                                ex[:, sl], start=(ki == 0),
                                stop=(ki == nki - 1))
                        den_ps = psD.tile([1, 512], F32, name="psD")
                        nc.tensor.matmul(den_ps[:], ones_col_b[:],
                                         den_acc[0][:], start=True, stop=False)
                        nc.tensor.matmul(den_ps[:], ones_col_b[:],
                                         den_acc[1][:], start=False, stop=True)
                        rec = pCe.tile([1, 512], F32, name="rec")
                        with nc.allow_low_precision(reason="softmax denom"):
                            nc.vector.reciprocal(rec[:], den_ps[:])
                        # broadcast on the (idle) Pool engine: one hop less
                        # than the PE outer-product + copy round-trip
                        bc_sb = pCe.tile([128, 512], F32, name="bc_sb")
                        nc.gpsimd.partition_broadcast(bc_sb[:], rec[:])
                        nc.vector.tensor_mul(oT_sb[h][:], oT_ps[:], bc_sb[:])

                    OB = 4                       # m-chunks per output DMA
                    for mg in range(c.DCH // OB):
                        ob = pCe.tile([128, OB, 512], BF16, name="ob")
                        for j in range(OB):
                            m = mg * OB + j
                            ps = psM.tile([128, 512], F32, name="psm")
                            for k in range(c.HPC):
                                nc.tensor.matmul(
                                    ps[:], wo_sb[k][:, m * 128:(m + 1) * 128],
                                    oT_sb[k][:], start=(k == 0),
                                    stop=(k == c.HPC - 1))
                            nc.scalar.copy(ob[:, j, :], ps[:])
                        nc.sync.dma_start(
                            out_d.ap()[mg * OB:(mg + 1) * OB, :, q0:q0 + 512]
                            .rearrange("m p c -> p m c"), ob[:])
            pBC_cm.__exit__(None, None, None)
    nc.compile()
    return nc


# --------------------------------------------------------------------------
# public entry point
# --------------------------------------------------------------------------

_CACHED = {}


def _get_nc(cfg):
    key = cfg
    if key not in _CACHED:
        _CACHED[key] = build(cfg)
    return _CACHED[key]


def kernel(hidden_states, Wq_a, q_a_ln_w, Wq_b, Wkv_a, kv_a_ln_w, Wkv_b, Wo):
    cfg = FULL
    in_maps = prep_inputs(cfg, hidden_states, Wq_a, q_a_ln_w, Wq_b, Wkv_a,
                          kv_a_ln_w, Wkv_b, Wo)
    nc = _get_nc(cfg)
    res = run_bass_kernel_spmd(nc, in_maps, core_ids=list(range(N_CORES)))
    acc = np.zeros((cfg.D, cfg.S), np.float32)
    for r in res.results:
        acc += np.asarray(r["outT"], np.float32).reshape(cfg.D, cfg.S)
    return np.ascontiguousarray(acc.T).reshape(1, cfg.S, cfg.D)
